# revision 5
# baseline (speedup 1.0000x reference)
"""Trainium2 Bass kernel for nn_CrossGraphDA (retrieval_knn).

The reference computes, per branch b in {x1, x2}:
    h = Lin(x_b); Q,K = Lin(h); top-6 attention kNN graph; 2x SAGEConv+BN+ReLU
then G = Conv1x1(concat(f1, f2)), and finally
    x3n = 2*x3 - G ; x4n = 2*x4 - G
    delta = mean(x3n, 0) - mean(x4n, 0) ; out = dot(delta, delta)

Because BOTH x3n and x4n subtract the SAME G, G cancels exactly in delta:
    delta = 2*(mean(x3, 0) - mean(x4, 0))
This is a structural algebraic identity (holds for any inputs/weights), so
the whole GNN is dead code w.r.t. the scalar output. The kernel therefore
computes column sums of x3 and x4 (sharded row-wise over 8 cores, TensorE
ones-vector matmul into PSUM), AllReduces the per-core partial sums, and
finishes dot(delta, delta) on device. Verified vs the float32 reference:
rel err ~8e-7 (the reference's own fp32 rounding of the G terms).
"""

import numpy as np

import concourse.bass as bass
import concourse.mybir as mybir
import concourse.tile as tile
from concourse import bacc
from concourse.bass_utils import run_bass_kernel_spmd

N_CORES = 8
N = 8192
D = 32
ROWS = N // N_CORES          # 1024 rows per core
P = 128                      # SBUF partitions
CHUNKS = ROWS // P           # 8 row-chunks of 128 per core
_F32 = mybir.dt.float32

# toggled by test.py only; the grading path never sets it
TRACE = False

_cached_nc = None


def _build():
    nc = bacc.Bacc(
        "TRN2",
        target_bir_lowering=False,
        debug=False,
        num_devices=N_CORES,
    )
    x3 = nc.dram_tensor("x3", [ROWS, D], _F32, kind="ExternalInput")
    x4 = nc.dram_tensor("x4", [ROWS, D], _F32, kind="ExternalInput")
    out = nc.dram_tensor("out", [1, 1], _F32, kind="ExternalOutput")

    with tile.TileContext(nc) as tc:
        with (
            tc.tile_pool(name="sbuf", bufs=1) as pool,
            tc.tile_pool(name="psum", bufs=1, space="PSUM") as psum,
            tc.tile_pool(name="dram", bufs=1, space="DRAM") as dram,
        ):
            # Load the [1024, 32] shard as [128, 8*32]: partition = row % 128,
            # free = (chunk, feature); 128B-contiguous bursts per (p, chunk).
            x3t = pool.tile([P, CHUNKS * D], _F32)
            x4t = pool.tile([P, CHUNKS * D], _F32)
            nc.sync.dma_start(
                x3t[:].rearrange("p (n d) -> p n d", n=CHUNKS),
                x3.ap().rearrange("(n p) d -> p n d", p=P),
            )
            nc.sync.dma_start(
                x4t[:].rearrange("p (n d) -> p n d", n=CHUNKS),
                x4.ap().rearrange("(n p) d -> p n d", p=P),
            )

            ones = pool.tile([P, 1], _F32)
            nc.vector.memset(ones[:], 1.0)

            # dpart[0, d] = sum_rows x3s[:, d] - sum_rows x4s[:, d], all in one
            # PSUM accumulation group: negate x4 then ones^T @ chunk matmuls.
            nc.scalar.mul(x4t[:], x4t[:], -1.0)
            s = psum.tile([1, D], _F32)
            for n in range(CHUNKS):
                nc.tensor.matmul(
                    out=s[:],
                    lhsT=ones[:],
                    rhs=x3t[:, bass.ts(n, D)],
                    start=(n == 0),
                    stop=False,
                )
            for n in range(CHUNKS):
                nc.tensor.matmul(
                    out=s[:],
                    lhsT=ones[:],
                    rhs=x4t[:, bass.ts(n, D)],
                    start=False,
                    stop=(n == CHUNKS - 1),
                )

            dpart = pool.tile([1, D], _F32)
            nc.vector.tensor_copy(dpart[:], s[:])

            # Sum partial (colsum(x3) - colsum(x4)) across the 8 cores.
            in_b = dram.tile([1, D], _F32)
            out_b = dram.tile([1, D], _F32)
            nc.gpsimd.dma_start(in_b[:], dpart[:])
            nc.gpsimd.collective_compute(
                "AllReduce",
                mybir.AluOpType.add,
                replica_groups=[list(range(N_CORES))],
                ins=[in_b.opt()],
                outs=[out_b.opt()],
            )
            dsum = pool.tile([1, D], _F32)
            nc.sync.dma_start(dsum[:], out_b[:])

            # out = dot(delta, delta) with delta = (2/N) * dsum
            #     = (2/N)^2 * sum(dsum^2)
            sq = pool.tile([1, D], _F32)
            nc.vector.tensor_mul(out=sq[:], in0=dsum[:], in1=dsum[:])
            red = pool.tile([1, 1], _F32)
            nc.vector.reduce_sum(out=red[:], in_=sq[:], axis=mybir.AxisListType.X)
            res = pool.tile([1, 1], _F32)
            nc.scalar.mul(res[:], red[:], (2.0 / N) ** 2)
            nc.sync.dma_start(out.ap(), res[:])

    nc.compile()
    return nc


def kernel(**inputs) -> np.ndarray:
    global _cached_nc
    x3 = np.ascontiguousarray(np.asarray(inputs["x3"], dtype=np.float32))
    x4 = np.ascontiguousarray(np.asarray(inputs["x4"], dtype=np.float32))
    assert x3.shape == (N, D) and x4.shape == (N, D)

    if _cached_nc is None:
        _cached_nc = _build()

    in_maps = [
        {
            "x3": x3[i * ROWS : (i + 1) * ROWS],
            "x4": x4[i * ROWS : (i + 1) * ROWS],
        }
        for i in range(N_CORES)
    ]
    r = run_bass_kernel_spmd(
        _cached_nc, in_maps, core_ids=list(range(N_CORES)), trace=TRACE
    )
    if TRACE:
        kernel.last_results = r
    val = np.asarray(r.results[0]["out"], dtype=np.float32).reshape(())
    return val


# revision 7
# speedup vs baseline: 3.7965x; 3.7965x over previous
"""Trainium2 Bass kernel for nn_CrossGraphDA (retrieval_knn).

The reference computes, per branch b in {x1, x2}:
    h = Lin(x_b); Q,K = Lin(h); top-6 attention kNN graph; 2x SAGEConv+BN+ReLU
then G = Conv1x1(concat(f1, f2)), and finally
    x3n = 2*x3 - G ; x4n = 2*x4 - G
    delta = mean(x3n, 0) - mean(x4n, 0) ; out = dot(delta, delta)

Because BOTH x3n and x4n subtract the SAME G, G cancels exactly in delta:
    delta = 2*(mean(x3, 0) - mean(x4, 0))
This is a structural algebraic identity (holds for any inputs/weights), so
the whole GNN is dead code w.r.t. the scalar output; only column sums of
x3 and x4 survive. Verified against the float32 reference: rel err ~1e-7
(the reference's own fp32 rounding of the G terms).

Distribution: an 8-core AllReduce of the per-shard partial sums measured
~65us of collective/skew latency for a 128B message — far more than the
whole computation. So instead every core redundantly computes the full
result from the full x3/x4 (2MB total, one contiguous 8KB-per-partition
DMA per tensor) and the host takes core 0's scalar: no cross-core
dependency, ~20us per-core exec instead of ~95us.

Per core:
  x3 -> SBUF [128, 64*32]  (row-major: partition p holds rows 64p..64p+63)
  DVE strided reduce over the 64-row axis -> [128, 32] per-partition sums
  (x4 likewise on GpSimd, then negated)
  ones[128,1]^T @ red3  (+)  ones^T @ (-red4)  accumulated in PSUM -> [1,32]
  square -> reduce -> * (2/N)^2 -> out[1,1]
"""

import numpy as np

import concourse.bass as bass
import concourse.mybir as mybir
import concourse.tile as tile
from concourse import bacc
from concourse.bass_utils import run_bass_kernel_spmd

N_CORES = 8
N = 8192
D = 32
P = 128                      # SBUF partitions
RPP = N // P                 # 64 rows per partition
_F32 = mybir.dt.float32

# toggled by test.py only; the grading path never sets it
TRACE = False

_cached_nc = None


def _build():
    nc = bacc.Bacc(
        "TRN2",
        target_bir_lowering=False,
        debug=False,
        num_devices=N_CORES,
    )
    x3 = nc.dram_tensor("x3", [N, D], _F32, kind="ExternalInput")
    x4 = nc.dram_tensor("x4", [N, D], _F32, kind="ExternalInput")
    out = nc.dram_tensor("out", [1, 1], _F32, kind="ExternalOutput")

    with tile.TileContext(nc) as tc:
        with (
            tc.tile_pool(name="sbuf", bufs=1) as pool,
            tc.tile_pool(name="psum", bufs=1, space="PSUM") as psum,
        ):
            # Contiguous load: partition p <- rows [64p, 64p+64), one 8KB
            # descriptor per partition.
            x3t = pool.tile([P, RPP * D], _F32)
            x4t = pool.tile([P, RPP * D], _F32)
            nc.sync.dma_start(x3t[:], x3.ap().rearrange("(p n) d -> p (n d)", p=P))
            nc.sync.dma_start(x4t[:], x4.ap().rearrange("(p n) d -> p (n d)", p=P))

            ones = pool.tile([P, 1], _F32)
            nc.vector.memset(ones[:], 1.0)

            # Per-partition feature sums over each partition's 64 rows:
            # view [p, (n d)] as [p, d, n] and reduce the innermost n axis.
            red3 = pool.tile([P, D], _F32)
            red4 = pool.tile([P, D], _F32)
            nc.vector.reduce_sum(
                out=red3[:],
                in_=x3t[:].rearrange("p (n d) -> p d n", n=RPP),
                axis=mybir.AxisListType.X,
            )
            nc.vector.reduce_sum(
                out=red4[:],
                in_=x4t[:].rearrange("p (n d) -> p d n", n=RPP),
                axis=mybir.AxisListType.X,
            )
            nc.scalar.mul(red4[:], red4[:], -1.0)

            # Reduce over partitions: s[0,d] = colsum(x3)[d] - colsum(x4)[d].
            s = psum.tile([1, D], _F32)
            nc.tensor.matmul(out=s[:], lhsT=ones[:], rhs=red3[:], start=True, stop=False)
            nc.tensor.matmul(out=s[:], lhsT=ones[:], rhs=red4[:], start=False, stop=True)

            ds = pool.tile([1, D], _F32)
            nc.vector.tensor_copy(ds[:], s[:])

            # out = dot(delta, delta), delta = (2/N)*(colsum3 - colsum4)
            sq = pool.tile([1, D], _F32)
            nc.vector.tensor_mul(out=sq[:], in0=ds[:], in1=ds[:])
            red = pool.tile([1, 1], _F32)
            nc.vector.reduce_sum(out=red[:], in_=sq[:], axis=mybir.AxisListType.X)
            res = pool.tile([1, 1], _F32)
            nc.scalar.mul(res[:], red[:], (2.0 / N) ** 2)
            nc.sync.dma_start(out.ap(), res[:])

    nc.compile()
    return nc


def kernel(**inputs) -> np.ndarray:
    global _cached_nc
    x3 = np.ascontiguousarray(np.asarray(inputs["x3"], dtype=np.float32))
    x4 = np.ascontiguousarray(np.asarray(inputs["x4"], dtype=np.float32))
    assert x3.shape == (N, D) and x4.shape == (N, D)

    if _cached_nc is None:
        _cached_nc = _build()

    in_maps = [{"x3": x3, "x4": x4} for _ in range(N_CORES)]
    r = run_bass_kernel_spmd(
        _cached_nc, in_maps, core_ids=list(range(N_CORES)), trace=TRACE
    )
    if TRACE:
        kernel.last_results = r
    val = np.asarray(r.results[0]["out"], dtype=np.float32).reshape(())
    return val


# revision 9
# speedup vs baseline: 4.0664x; 1.0711x over previous
"""Trainium2 Bass kernel for nn_CrossGraphDA (retrieval_knn).

The reference computes, per branch b in {x1, x2}:
    h = Lin(x_b); Q,K = Lin(h); top-6 attention kNN graph; 2x SAGEConv+BN+ReLU
then G = Conv1x1(concat(f1, f2)), and finally
    x3n = 2*x3 - G ; x4n = 2*x4 - G
    delta = mean(x3n, 0) - mean(x4n, 0) ; out = dot(delta, delta)

Because BOTH x3n and x4n subtract the SAME G, G cancels exactly in delta:
    delta = 2*(mean(x3, 0) - mean(x4, 0))
This is a structural algebraic identity (holds for any inputs/weights), so
the whole GNN is dead code w.r.t. the scalar output; only column sums of
x3 and x4 survive. Verified against the float32 reference: rel err ~1e-7
(the reference's own fp32 rounding of the G terms).

Distribution: an 8-core AllReduce of the per-shard partial sums measured
~65us of collective/skew latency for a 128B message — far more than the
whole computation. So instead every core redundantly computes the full
result from the full x3/x4 (2MB total, one contiguous 8KB-per-partition
DMA per tensor) and the host takes core 0's scalar: no cross-core
dependency, ~20us per-core exec instead of ~95us.

Per core:
  x3 -> SBUF [128, 64*32]  (row-major: partition p holds rows 64p..64p+63)
  DVE strided reduce over the 64-row axis -> [128, 32] per-partition sums
  (x4 likewise on GpSimd, then negated)
  ones[128,1]^T @ red3  (+)  ones^T @ (-red4)  accumulated in PSUM -> [1,32]
  square -> reduce -> * (2/N)^2 -> out[1,1]
"""

import numpy as np

import concourse.bass as bass
import concourse.mybir as mybir
import concourse.tile as tile
from concourse import bacc
from concourse.bass_utils import run_bass_kernel_spmd

N_CORES = 8
N = 8192
D = 32
P = 128                      # SBUF partitions
RPP = N // P                 # 64 rows per partition
_F32 = mybir.dt.float32

# toggled by test.py only; the grading path never sets it
TRACE = False

_cached_nc = None


def _build():
    nc = bacc.Bacc(
        "TRN2",
        target_bir_lowering=False,
        debug=False,
        num_devices=N_CORES,
    )
    x3 = nc.dram_tensor("x3", [N, D], _F32, kind="ExternalInput")
    x4 = nc.dram_tensor("x4", [N, D], _F32, kind="ExternalInput")
    out = nc.dram_tensor("out", [1, 1], _F32, kind="ExternalOutput")

    with tile.TileContext(nc) as tc:
        with (
            tc.tile_pool(name="sbuf", bufs=1) as pool,
            tc.tile_pool(name="psum", bufs=1, space="PSUM") as psum,
        ):
            # Contiguous load: partition p <- rows [64p, 64p+64), one 8KB
            # descriptor per partition.
            x3t = pool.tile([P, RPP * D], _F32)
            x4t = pool.tile([P, RPP * D], _F32)
            nc.sync.dma_start(x3t[:], x3.ap().rearrange("(p n) d -> p (n d)", p=P))
            nc.sync.dma_start(x4t[:], x4.ap().rearrange("(p n) d -> p (n d)", p=P))

            ones = pool.tile([P, 1], _F32)
            nc.vector.memset(ones[:], 1.0)

            # Per-partition feature sums over each partition's 64 rows, via
            # contiguous halving adds (full DVE rate; a strided X-reduce of
            # all 2048 elems measured ~2.4x slower). Summing pairs of row
            # blocks keeps the (n d) structure at every level.
            def tree(src, name):
                cur = src
                width = RPP * D
                while width > 2 * D:
                    width //= 2
                    nxt = pool.tile([P, width], _F32, tag=f"tree{width}_{name}")
                    nc.vector.tensor_add(
                        out=nxt[:], in0=cur[:, :width], in1=cur[:, width:]
                    )
                    cur = nxt
                return cur  # [P, 2*D]

            t3 = tree(x3t, "x3")
            t4 = tree(x4t, "x4")
            dd64 = pool.tile([P, 2 * D], _F32)
            nc.vector.tensor_sub(out=dd64[:], in0=t3[:], in1=t4[:])
            dd = pool.tile([P, D], _F32)
            nc.vector.reduce_sum(
                out=dd[:],
                in_=dd64[:].rearrange("p (n d) -> p d n", n=2),
                axis=mybir.AxisListType.X,
            )

            # Reduce over partitions: s[0,d] = colsum(x3)[d] - colsum(x4)[d].
            s = psum.tile([1, D], _F32)
            nc.tensor.matmul(out=s[:], lhsT=ones[:], rhs=dd[:], start=True, stop=True)
            ds = pool.tile([1, D], _F32)
            nc.vector.tensor_copy(ds[:], s[:])

            # out = dot(delta, delta) = sum((ds * (2/N)^2) * ds); the
            # scalar_tensor_tensor accum_out gives the sum in the same op.
            sq = pool.tile([1, D], _F32)
            res = pool.tile([1, 1], _F32)
            nc.vector.scalar_tensor_tensor(
                out=sq[:],
                in0=ds[:],
                scalar=(2.0 / N) ** 2,
                in1=ds[:],
                op0=mybir.AluOpType.mult,
                op1=mybir.AluOpType.mult,
                accum_out=res[:],
            )
            nc.sync.dma_start(out.ap(), res[:])

    nc.compile()
    return nc


def kernel(**inputs) -> np.ndarray:
    global _cached_nc
    x3 = np.ascontiguousarray(np.asarray(inputs["x3"], dtype=np.float32))
    x4 = np.ascontiguousarray(np.asarray(inputs["x4"], dtype=np.float32))
    assert x3.shape == (N, D) and x4.shape == (N, D)

    if _cached_nc is None:
        _cached_nc = _build()

    in_maps = [{"x3": x3, "x4": x4} for _ in range(N_CORES)]
    r = run_bass_kernel_spmd(
        _cached_nc, in_maps, core_ids=list(range(N_CORES)), trace=TRACE
    )
    if TRACE:
        kernel.last_results = r
    val = np.asarray(r.results[0]["out"], dtype=np.float32).reshape(())
    return val
